# revision 72
# baseline (speedup 1.0000x reference)
"""BitNet transformer block on 8 Trainium2 NeuronCores (Bass/Tile).

Sharding: DP2 (batch) x TP4 (Megatron-style, sequence-parallel norms).
Cores 0-3 -> batch 0, cores 4-7 -> batch 1. Within each group of 4:
  - weights are ternarized on the host (exact {-1,0,1} in bf16) and the
    four per-tensor dequant scales (mean|w|/127) ride in as a tiny input,
  - each core owns 512 tokens for LN + act_quant (sequence parallel);
    quantized activations (small exact ints carried as bf16) are
    AllGathered, making every matmul an exact integer matmul in bf16
    with fp32 PSUM accumulation,
  - attention is head-parallel (4 heads/core) in S^T layout: exp with no
    max subtraction (scores are O(1)); P^T feeds O^T = v^T @ P^T directly;
    a ones column appended to v yields the softmax denominator,
  - proj is row-parallel: raw integer partial sums ReduceScatter in
    bf16 and are dequantized after the reduce,
  - fc1/fc2 are sequence-parallel with full ternary weights streamed
    just-in-time from dedicated double-buffered pools.
"""

import sys

for _p in ("/opt/trn_rl_repo",):
    if _p not in sys.path:
        sys.path.append(_p)

import numpy as np
import ml_dtypes

BF16NP = ml_dtypes.bfloat16
_BASS = {}


def _imports():
    if _BASS:
        return _BASS
    import concourse.bass as bass
    import concourse.bass_isa as bass_isa
    import concourse.mybir as mybir
    import concourse.tile as tile
    from concourse import bacc
    from concourse.bass_utils import run_bass_kernel_spmd
    from concourse.masks import make_identity
    _BASS.update(bass=bass, bass_isa=bass_isa, mybir=mybir, tile=tile,
                 bacc=bacc, run=run_bass_kernel_spmd, mkid=make_identity)
    return _BASS

# ---- problem constants (hardcoded per spec) ----
B, N, C, H = 2, 2048, 1024, 16
HID = 4 * C
NCORES, TP = 8, 4
TOK = N // TP            # 512 tokens per core
TT_LOC = TOK // 128      # 4
TT_ALL = N // 128        # 16
HPC = H // TP            # 4 heads per core
DH = C // H              # 64
CS = C // TP             # 256 channel shard (proj contraction)
P = 128
KT = C // P              # 8
HKT = HID // P           # 32 fc2 contraction k-tiles
HC = HID // 512          # 8 fc1 hidden col chunks
EPS = 1e-5
MAGIC = 12582912.0       # 1.5 * 2**23: fp32 round-half-even trick
G4 = [[0, 1, 2, 3], [4, 5, 6, 7]]


def build_kernel(g1_trivial, g2_trivial):
    m = _imports()
    bass, bass_isa, mybir, tile, bacc = (m["bass"], m["bass_isa"], m["mybir"],
                                         m["tile"], m["bacc"])
    F32, BF16 = mybir.dt.float32, mybir.dt.bfloat16
    AX, ALU, ACTF = (mybir.AxisListType, mybir.AluOpType,
                     mybir.ActivationFunctionType)

    make_identity = m["mkid"]
    nc = bacc.Bacc("TRN2", target_bir_lowering=False, debug=False,
                   num_devices=NCORES)

    x_sh = nc.dram_tensor("x_sh", [TOK, C], F32, kind="ExternalInput")
    x_full = nc.dram_tensor("x_full", [N, C], F32, kind="ExternalInput")
    wqkT = nc.dram_tensor("wqkT", [C, 2 * CS], BF16, kind="ExternalInput")
    wvT = nc.dram_tensor("wvT", [C, CS], BF16, kind="ExternalInput")
    wpT = nc.dram_tensor("wpT", [C, C], BF16, kind="ExternalInput")
    w1T = nc.dram_tensor("w1T", [C, HID], BF16, kind="ExternalInput")
    w2T = nc.dram_tensor("w2T", [HID, C], BF16, kind="ExternalInput")
    scl = nc.dram_tensor("scl", [4], F32, kind="ExternalInput")
    bqk = nc.dram_tensor("bqk", [2 * CS], F32, kind="ExternalInput")
    bv = nc.dram_tensor("bv", [CS], F32, kind="ExternalInput")
    bp = nc.dram_tensor("bp", [C], F32, kind="ExternalInput")
    bf1 = nc.dram_tensor("bf1", [HID], F32, kind="ExternalInput")
    bf2 = nc.dram_tensor("bf2", [C], F32, kind="ExternalInput")
    g1 = be1 = g2 = be2 = None
    if not g1_trivial:
        g1 = nc.dram_tensor("g1", [C], F32, kind="ExternalInput")
        be1 = nc.dram_tensor("be1", [C], F32, kind="ExternalInput")
    if not g2_trivial:
        g2 = nc.dram_tensor("g2", [C], F32, kind="ExternalInput")
        be2 = nc.dram_tensor("be2", [C], F32, kind="ExternalInput")
    onehot = nc.dram_tensor("onehot", [TP], F32, kind="ExternalInput")
    y_sh = nc.dram_tensor("y_sh", [TOK, C], F32, kind="ExternalOutput")

    with tile.TileContext(nc) as tc:
        import contextlib
        with contextlib.ExitStack() as ctx:
            dram = ctx.enter_context(tc.tile_pool(name="dram", bufs=1, space="DRAM"))
            consts = ctx.enter_context(tc.tile_pool(name="consts", bufs=1))
            wres = ctx.enter_context(tc.tile_pool(name="wres", bufs=1))
            acts = ctx.enter_context(tc.tile_pool(name="acts", bufs=1))
            big = ctx.enter_context(tc.tile_pool(name="big", bufs=1))
            rowp = ctx.enter_context(tc.tile_pool(name="rowp", bufs=1))
            w1p = ctx.enter_context(tc.tile_pool(name="w1p", bufs=2))
            w2p = ctx.enter_context(tc.tile_pool(name="w2p", bufs=3))
            t8 = ctx.enter_context(tc.tile_pool(name="t8", bufs=2))
            t4 = ctx.enter_context(tc.tile_pool(name="t4", bufs=2))
            t2 = ctx.enter_context(tc.tile_pool(name="t2", bufs=2))
            t1 = ctx.enter_context(tc.tile_pool(name="t1", bufs=3))
            brow = ctx.enter_context(tc.tile_pool(name="brow", bufs=2))
            sm = ctx.enter_context(tc.tile_pool(name="sm", bufs=2))
            psp = ctx.enter_context(tc.tile_pool(name="psp", bufs=2, space="PSUM"))
            psa = ctx.enter_context(tc.tile_pool(name="psa", bufs=1, space="PSUM"))

            # ---------- DRAM internal buffers ----------
            def dt(name, shape, dtype):
                return dram.tile(shape, dtype, name=name)

            HTOK = TOK // 2
            q1_dram = dt("q1_dram", [N, C], BF16)
            g_dram = dt("g_dram", [TOK, HID], BF16)
            l_dram = dt("l_dram", [HPC, N], F32)
            rf_dram = dt("rf_dram", [TT_ALL * HPC * P], F32)
            xmid_dram = dt("xmid_dram", [TOK, C], F32)
            ago_in = [dt("ago_in0", [N], F32), dt("ago_in1", [N], F32)]
            ago_out = [dt("ago_out0", [TP * N], F32),
                       dt("ago_out1", [TP * N], F32)]
            # AllGather of quantized o (channel-major [q, p, t] per rank),
            # chunked in two token halves for pipelining
            agq_in = [dt("agq_in0", [2 * P * (N // 2)], BF16),
                      dt("agq_in1", [2 * P * (N // 2)], BF16)]
            agq_out = [dt("agq_out0", [TP * 2 * P * (N // 2)], BF16),
                       dt("agq_out1", [TP * 2 * P * (N // 2)], BF16)]

            # ---------- constants / bias rows ----------
            eps_col = consts.tile([P, 1], F32, name="eps_col")
            nc.vector.memset(eps_col[:], EPS)
            magic_col = consts.tile([P, 1], F32, name="magic_col")
            nc.vector.memset(magic_col[:], MAGIC)
            # S2[k, i] = 1 if (i // 64) == k else 0 — head-half selector
            _s2 = np.zeros((2, P), np.float32)
            _s2[0, 0:DH] = 1.0
            _s2[1, DH:P] = 1.0
            sel2_dram = nc.inline_tensor(_s2.reshape(-1), "sel2_dram")
            sel2 = consts.tile([2, P], F32, name="sel2")
            nc.sync.dma_start(sel2[:],
                              sel2_dram[:].rearrange("(k p) -> k p", k=2))
            ident = consts.tile([P, P], F32, name="ident")
            make_identity(nc, ident[:])
            ident_bf = consts.tile([P, P], BF16, name="ident_bf")
            make_identity(nc, ident_bf[:])
            ones_row = consts.tile([1, P], F32, name="ones_row")
            nc.vector.memset(ones_row[:], 1.0)

            def bcast_row(dram_ap, n, name, pool=None, tag=None):
                if pool is None:
                    r = consts.tile([P, n], F32, name=name)
                else:
                    r = pool.tile([P, 1024], F32, name=name, tag=tag or "brow")[:, :n]
                nc.sync.dma_start(r[:], dram_ap[None, :].to_broadcast((P, n)))
                return r

            bv_row = bcast_row(bv[:], CS, "bv_row")
            bqk_col = consts.tile([P, 4], F32, name="bqk_col")
            nc.sync.dma_start(bqk_col[:], bqk[:].rearrange("(j p) -> p j", p=P))
            oh_bc = consts.tile([P, TP], F32, name="oh_bc")
            nc.sync.dma_start(oh_bc[:], onehot[None, :].to_broadcast((P, TP)))
            # per-tensor dequant scales (mean|w|/127), broadcast to all rows
            mean_bc = consts.tile([P, 4], F32, name="mean_bc")
            nc.sync.dma_start(mean_bc[:], scl[None, :].to_broadcast((P, 4)))

            # ---------- resident ternary weights (attention path) ----------
            wqk_bf = wres.tile([P, KT, 2 * CS], BF16, name="wqk_bf")
            nc.gpsimd.dma_start(wqk_bf[:],
                                wqkT[:].rearrange("(o p) c -> p o c", p=P))
            wv_bf = wres.tile([P, KT, CS], BF16, name="wv_bf")
            nc.gpsimd.dma_start(wv_bf[:],
                                wvT[:].rearrange("(o p) c -> p o c", p=P))
            wp_bf = wres.tile([P, KT, C], BF16, name="wp_bf")
            nc.gpsimd.dma_start(wp_bf[:],
                                wpT[:].rearrange("(o p) c -> p o c", p=P))

            def own_select(dst, col_g):
                # dst[P, TT_LOC] = rank-selected block of col_g[P, TT_ALL]
                tmp_os = sm.tile([P, TT_LOC], F32, tag="ownsel")
                for r in range(TP):
                    src = col_g[:, TT_LOC * r:TT_LOC * (r + 1)]
                    if r == 0:
                        nc.vector.tensor_scalar(dst, src, oh_bc[:, 0:1], None,
                                                op0=ALU.mult)
                    else:
                        nc.vector.tensor_scalar(tmp_os[:], src,
                                                oh_bc[:, r:r + 1], None,
                                                op0=ALU.mult)
                        nc.vector.tensor_tensor(dst, dst, tmp_os[:], ALU.add)

            # ---------- LN + act_quant (normalize on DVE or ACT per tile,
            # quantize-mult on ACT, round-sub on DVE) ----------
            def ln_quant(x_tile, g_row, be_row, trivial, qout_bf, m_out,
                         mode="act"):
                st6 = sm.tile([P, 2, 6], F32, tag="bnst")
                nc.vector.bn_stats(st6[:, 0, :], x_tile[:, 0:C // 2])
                nc.vector.bn_stats(st6[:, 1, :], x_tile[:, C // 2:C])
                agg = sm.tile([P, 2], F32, tag="bnagg")
                nc.vector.bn_aggr(agg[:], st6[:])
                rstd = sm.tile([P, 1], F32, tag="rstd")
                nc.scalar.activation(rstd[:], agg[:, 1:2], ACTF.Sqrt, bias=eps_col[:])
                nc.vector.reciprocal(rstd[:], rstd[:])
                h = t4.tile([P, C], F32, tag="t4f32")
                if mode == "act":
                    negmu = sm.tile([P, 1], F32, tag="negmu")
                    nc.vector.tensor_scalar(negmu[:], agg[:, 0:1], -1.0, None,
                                            op0=ALU.mult)
                    nc.vector.tensor_scalar(h[:], x_tile, negmu[:], None,
                                            op0=ALU.add)
                    nc.scalar.activation(h[:], h[:], ACTF.Copy, bias=0.0,
                                         scale=rstd[:])
                else:
                    nc.vector.tensor_scalar(h[:], x_tile, agg[:, 0:1],
                                            rstd[:], op0=ALU.subtract,
                                            op1=ALU.mult)
                if not trivial:
                    nc.vector.tensor_tensor(h[:], h[:], g_row[:, :C], ALU.mult)
                    nc.vector.tensor_tensor(h[:], h[:], be_row[:, :C], ALU.add)
                nc.vector.tensor_reduce(m_out, h[:], axis=AX.X, op=ALU.max,
                                        apply_absolute_value=True)
                nc.vector.tensor_scalar(m_out, m_out, EPS, None, op0=ALU.max)
                s = sm.tile([P, 1], F32, tag="qs")
                nc.vector.reciprocal(s[:], m_out)
                nc.vector.tensor_scalar(s[:], s[:], 127.0, None, op0=ALU.mult)
                if mode == "act":
                    nc.scalar.activation(h[:], h[:], ACTF.Copy, bias=MAGIC,
                                         scale=s[:])
                else:
                    nc.vector.tensor_scalar(h[:], h[:], s[:], None,
                                            op0=ALU.mult)
                    nc.vector.tensor_scalar(h[:], h[:], MAGIC, None,
                                            op0=ALU.add)
                nc.vector.tensor_scalar(qout_bf, h[:], MAGIC, None,
                                        op0=ALU.subtract)

            g1_row = be1_row = None
            if not g1_trivial:
                g1_row = bcast_row(g1[:], C, "g1_row", pool=brow)
                be1_row = bcast_row(be1[:], C, "be1_row", pool=brow)
            # Replicated LN1: every core norms+quantizes all 2048 tokens of
            # its batch locally (no collective on the critical path).
            # q1T built by PE transposes inline; per-token scale rows built
            # by tiny [P,1]->[1,128] PE transposes + broadcast matmuls.
            m1_col = sm.tile([P, TT_ALL], F32, name="m1_col")
            rinv1_bc = rowp.tile([P, N], F32, tag="rowtmp")
            m1row = rowp.tile([1, N], F32, tag="m1row", name="m1row")
            rinv1_col = sm.tile([P, TT_ALL], F32, name="rinv1_col")
            qk_bf = acts.tile([P, 4, N], BF16, tag="gqk", name="qk_bf")
            v_aug = acts.tile([P, TT_ALL, HPC, DH + 1], BF16, tag="vaug",
                              name="v_aug")
            nc.vector.memset(v_aug[:, :, :, DH:DH + 1], 1.0)

            def qk_cols(q1T, rb, t1c, jts):
                sl = slice(t1c * 512, (t1c + 1) * 512)
                for jt in jts:
                    pqk = psp.tile([P, 512], F32, tag="pb")
                    for ct in range(KT):
                        nc.tensor.matmul(pqk[:], wqk_bf[:, ct, jt * P:(jt + 1) * P],
                                         q1T[:, ct, :], start=(ct == 0),
                                         stop=(ct == KT - 1))
                    dq = t2.tile([P, 512], F32, tag="t2f32")
                    nc.vector.tensor_tensor(dq[:], pqk[:], rb, ALU.mult)
                    nc.vector.tensor_scalar(qk_bf[:, jt, sl], dq[:],
                                            bqk_col[:, jt:jt + 1], None,
                                            op0=ALU.add)

            q1Ts = []
            for t1c in range(4):
                q1T = t8.tile([P, KT, 512], BF16, tag="t8bf", bufs=2,
                              name="q1T%d" % t1c)
                for jj in range(TT_LOC):
                    j = t1c * TT_LOC + jj
                    xt = t4.tile([P, C], F32, tag="xload", bufs=2)
                    nc.sync.dma_start(xt[:], x_full[j * P:(j + 1) * P, :])
                    q1t = t2.tile([P, C], BF16, tag="t2bf")
                    ln_quant(xt[:], g1_row, be1_row, g1_trivial, q1t[:],
                             m1_col[:, j:j + 1],
                             mode=("act" if j % 2 == 0 else "dve"))
                    weng = [nc.gpsimd, nc.sync][j % 2]
                    weng.dma_start(q1_dram[j * P:(j + 1) * P, :], q1t[:])
                    teng = [nc.sync, nc.scalar][j % 2]
                    teng.dma_start_transpose(
                        q1T[:, :, jj * P:(jj + 1) * P],
                        q1_dram[j * P:(j + 1) * P, :])
                    m1t_ps = psp.tile([P, P], F32, tag="pb", name="m1t_ps")
                    nc.tensor.transpose(m1t_ps[0:1, 0:P],
                                        m1_col[:, j:j + 1], ident[:])
                    nc.vector.tensor_copy(m1row[:, j * P:(j + 1) * P],
                                          m1t_ps[0:1, 0:P])
                q1Ts.append(q1T)
                sl = slice(t1c * 512, (t1c + 1) * 512)
                rbc_ps = psp.tile([P, 512], F32, tag="pb", name="rbc_ps")
                nc.tensor.matmul(rbc_ps[:], ones_row[:], m1row[:, sl],
                                 start=True, stop=True)
                nc.vector.tensor_scalar(rinv1_bc[:, sl], rbc_ps[:],
                                        mean_bc[:, 0:1], None, op0=ALU.mult)
                nc.vector.tensor_scalar(
                    rinv1_col[:, t1c * 4:(t1c + 1) * 4],
                    m1_col[:, t1c * 4:(t1c + 1) * 4],
                    mean_bc[:, 0:1], None, op0=ALU.mult)
                # qkv for this token slice right away (k first, then v, q)
                qk_cols(q1T, rinv1_bc[:, sl], t1c, (2, 3))
                for k in range(4):
                    tt = t1c * 4 + k
                    pv = psp.tile([P, 512], F32, tag="pb")
                    for ct in range(KT):
                        nc.tensor.matmul(pv[:, 0:CS],
                                         q1T[:, ct, k * P:(k + 1) * P],
                                         wv_bf[:, ct, :], start=(ct == 0),
                                         stop=(ct == KT - 1))
                    vdq = t1.tile([P, CS], F32, tag="t1f32")
                    nc.vector.tensor_scalar(vdq[:], pv[:, 0:CS],
                                            rinv1_col[:, tt:tt + 1], None,
                                            op0=ALU.mult)
                    nc.vector.tensor_tensor(
                        v_aug[:, tt, :, 0:DH],
                        vdq[:].rearrange("p (h d) -> p h d", d=DH),
                        bv_row[:].rearrange("p (h d) -> p h d", d=DH), ALU.add)
                qk_cols(q1T, rinv1_bc[:, sl], t1c, (0, 1))

            # ---------- attention ----------
            o_un = big.tile([P, HPC // 2, N], BF16, tag="bigf32")
            moc = sm.tile([P, TT_ALL, HPC], F32, name="moc")
            lcol = sm.tile([P, TT_ALL, HPC], F32, name="lcol")
            SCALE = DH ** -0.5
            for hp in range(HPC // 2):
                h_e, h_o = 2 * hp, 2 * hp + 1
                for t1c in range(4):
                    sl = slice(t1c * 512, (t1c + 1) * 512)
                    po_e = psa.tile([P, 512], F32, tag="po_e")
                    po_o = psa.tile([P, 512], F32, tag="po_o")
                    for tt2 in range(TT_ALL):
                        sreg = psp.tile([P, 2, 512], F32, tag="sreg", bufs=2)
                        for ii, hh in enumerate((h_e, h_o)):
                            jk = CS + DH * hh
                            jq = DH * hh
                            kT_ap = qk_bf[(jk % P):(jk % P) + DH, jk // P,
                                          tt2 * P:(tt2 + 1) * P]
                            qT_ap = qk_bf[(jq % P):(jq % P) + DH, jq // P, sl]
                            nc.tensor.matmul(sreg[:, ii, :], kT_ap, qT_ap,
                                             start=True, stop=True)
                        pt = t1.tile([P, 2, 512], BF16, tag="ptbf", bufs=4)
                        nc.scalar.activation(pt[:], sreg[:], ACTF.Exp, scale=SCALE)
                        nc.tensor.matmul(po_e[0:DH + 1, :], v_aug[:, tt2, h_e, :],
                                         pt[:, 0, :], start=(tt2 == 0),
                                         stop=(tt2 == TT_ALL - 1),
                                         skip_group_check=True)
                        nc.tensor.matmul(po_o[0:DH + 1, :], v_aug[:, tt2, h_o, :],
                                         pt[:, 1, :], start=(tt2 == 0),
                                         stop=(tt2 == TT_ALL - 1),
                                         skip_group_check=True)
                    nc.vector.tensor_copy(o_un[0:DH, hp, sl], po_e[0:DH, :])
                    nc.vector.tensor_copy(o_un[DH:2 * DH, hp, sl], po_o[0:DH, :])
                    lr = t2.tile([P, 512], F32, tag="t2f32")
                    nc.vector.tensor_copy(lr[DH:DH + 1, :], po_e[DH:DH + 1, :])
                    lr2 = t2.tile([P, 512], F32, tag="t2f32")
                    nc.vector.tensor_copy(lr2[DH:DH + 1, :], po_o[DH:DH + 1, :])
                    nc.sync.dma_start(l_dram[h_e, sl], lr[DH:DH + 1, :])
                    nc.sync.dma_start(l_dram[h_o, sl], lr2[DH:DH + 1, :])
                # per-pair absmax stats as soon as the pair finishes
                for tb in range(TT_ALL):
                    tr_ps = psp.tile([P, P], BF16, tag="pb")
                    nc.tensor.transpose(tr_ps[:, 0:P],
                                        o_un[:, hp, tb * P:(tb + 1) * P],
                                        ident_bf[:])
                    nc.vector.tensor_reduce(
                        moc[:, tb, 2 * hp:2 * hp + 2],
                        tr_ps[:, 0:P].rearrange("p (h d) -> p h d", d=DH),
                        axis=AX.X, op=ALU.max, apply_absolute_value=True)
                # normalize stats + AllGather this pair's token maxima now
                for hh in (h_e, h_o):
                    nc.sync.dma_start(
                        lcol[:, :, hh],
                        l_dram[hh, :].rearrange("(j p) -> p j", p=P))
                lsl = lcol[:, :, 2 * hp:2 * hp + 2]
                nc.vector.reciprocal(lsl, lsl)
                msl = moc[:, :, 2 * hp:2 * hp + 2]
                nc.vector.tensor_tensor(msl, msl, lsl, ALU.mult)
                mo_p = sm.tile([P, TT_ALL], F32, tag="mo_p")
                nc.vector.tensor_reduce(mo_p[:], msl, axis=AX.X, op=ALU.max)
                nc.vector.tensor_scalar(mo_p[:], mo_p[:], EPS, None,
                                        op0=ALU.max)
                nc.sync.dma_start(
                    ago_in[hp][:].rearrange("(j p) -> p j", p=P), mo_p[:])
                nc.gpsimd.collective_compute(
                    "AllGather", ALU.bypass, replica_groups=G4,
                    ins=[ago_in[hp].opt()], outs=[ago_out[hp].opt()])

            # ---------- global o absmax ----------
            mo_all = sm.tile([P, TT_ALL, 2 * TP], F32, name="mo_all")
            for hp in range(2):
                for r in range(TP):
                    nc.sync.dma_start(
                        mo_all[:, :, hp * TP + r],
                        ago_out[hp][r * N:(r + 1) * N]
                        .rearrange("(j p) -> p j", p=P))
            mo_colg = sm.tile([P, TT_ALL], F32, name="mo_colg")
            nc.vector.tensor_reduce(mo_colg[:], mo_all[:], axis=AX.X, op=ALU.max)

            so_col = sm.tile([P, TT_ALL], F32, name="so_col")
            nc.vector.reciprocal(so_col[:], mo_colg[:])
            nc.vector.tensor_scalar(so_col[:], so_col[:], 127.0, None,
                                    op0=ALU.mult)
            # rowf[t, h] = so[t] / l_h[t] (col space); PE-transpose to rows
            # and bounce through DRAM contiguously (flat idx (tb*HPC+h)*P + p)
            rowf_col = sm.tile([P, TT_ALL, HPC], F32, name="rowf_col")
            nc.vector.tensor_tensor(rowf_col[:], lcol[:],
                                    so_col[:, :, None].to_broadcast(
                                        (P, TT_ALL, HPC)), ALU.mult)
            rfT_ps = psp.tile([P, P], F32, tag="pb", name="rfT_ps")
            nc.tensor.transpose(rfT_ps[0:TT_ALL * HPC, 0:P],
                                rowf_col[:].rearrange("p j h -> p (j h)"),
                                ident[:])
            rfT_sb = t1.tile([TT_ALL * HPC, P], F32, tag="t1f32",
                             name="rfT_sb")
            nc.vector.tensor_copy(rfT_sb[:], rfT_ps[0:TT_ALL * HPC, 0:P])
            nc.sync.dma_start(rf_dram[:].rearrange("(q p) -> q p", p=P),
                              rfT_sb[:])
            rf_rows = rf_dram[:].rearrange("(b h p) -> h b p", h=HPC, p=P)

            # ---------- quantize o + AllToAll (chunked by token block) ----------
            oq = acts.tile([P, HPC // 2, N], BF16, tag="oq8", name="oq")
            for ch in range(4):
                csl = slice(ch * 512, (ch + 1) * 512)
                for q in range(2):
                    h_e, h_o = 2 * q, 2 * q + 1
                    rfr2 = sm.tile([2, 512], F32, tag="rfr", name="rfr2")
                    nc.sync.dma_start(
                        rfr2[0:1, :].rearrange("one (b p) -> one b p", p=P),
                        rf_rows[h_e:h_e + 1, ch * 4:(ch + 1) * 4, :])
                    nc.sync.dma_start(
                        rfr2[1:2, :].rearrange("one (b p) -> one b p", p=P),
                        rf_rows[h_o:h_o + 1, ch * 4:(ch + 1) * 4, :])
                    bc_ps = psp.tile([P, 512], F32, tag="pb")
                    nc.tensor.matmul(bc_ps[:], sel2[:], rfr2[:],
                                     start=True, stop=True)
                    tq = t2.tile([P, 512], F32, tag="t2f32")
                    nc.vector.tensor_tensor(tq[:], o_un[:, q, csl],
                                            bc_ps[:], ALU.mult)
                    nc.vector.tensor_scalar(oq[:, q, csl], tq[:],
                                            MAGIC, MAGIC,
                                            op0=ALU.add, op1=ALU.subtract)
                for q in range(2):
                    nc.gpsimd.dma_start(
                        agq_in[ch // 2][:].rearrange(
                            "(q p t) -> q p t", q=2,
                            p=P)[q, :, (ch % 2) * 512:(ch % 2 + 1) * 512],
                        oq[:, q, csl])
                if ch % 2 == 1:
                    nc.gpsimd.collective_compute(
                        "AllGather", ALU.bypass, replica_groups=G4,
                        ins=[agq_in[ch // 2].opt()],
                        outs=[agq_out[ch // 2].opt()])
            # oqa[ct] = channels ct*128..+127 of quantized o for OWN tokens:
            # onehot-select the own token chunk from the gathered halves
            oqa = acts.tile([P, KT, 512], BF16, tag="oq8", name="oqa")
            engs4 = [nc.sync, nc.scalar, nc.gpsimd, nc.sync]
            tsel = t8.tile([P, KT, 512], BF16, tag="tsel", bufs=1,
                           name="tsel")
            for c in range(TP):
                agv = agq_out[c // 2][:].rearrange("(ct p t) -> ct p t",
                                                   ct=2 * TP, p=P)
                cnd = t8.tile([P, KT, 512], BF16, tag="t8bf", bufs=2,
                              name="cnd%d" % c)
                engs4[c].dma_start(
                    cnd[:],
                    agv[:, :, (c % 2) * 512:(c % 2 + 1) * 512]
                    .rearrange("ct p t -> p ct t"))
                if c == 0:
                    nc.vector.tensor_scalar(oqa[:], cnd[:], oh_bc[:, 0:1],
                                            None, op0=ALU.mult)
                else:
                    nc.vector.tensor_scalar(tsel[:], cnd[:],
                                            oh_bc[:, c:c + 1], None,
                                            op0=ALU.mult)
                    nc.vector.tensor_tensor(oqa[:], oqa[:], tsel[:], ALU.add)

            # ---------- proj (column-parallel, local, exact) ----------
            rinvo_own = sm.tile([P, TT_LOC], F32, name="rinvo_own")
            own_select(rinvo_own[:], mo_colg[:])
            nc.vector.tensor_scalar(rinvo_own[:], rinvo_own[:],
                                    mean_bc[:, 1:2], None, op0=ALU.mult)
            bp_row = bcast_row(bp[:], C, "bp_row", pool=brow)
            g2_row = be2_row = None
            if not g2_trivial:
                g2_row = bcast_row(g2[:], C, "g2_row")
                be2_row = bcast_row(be2[:], C, "be2_row")
            m2_loc = sm.tile([P, TT_LOC], F32, name="m2_loc")
            q2T = acts.tile([P, KT, TOK], BF16, tag="vaug", name="q2T")
            for j in range(TT_LOC):
                pp_pair = psp.tile([P, 2, 512], F32, tag="sreg",
                                   name="pp_pair")
                for cc in range(2):
                    for ct in range(KT):
                        nc.tensor.matmul(pp_pair[:, cc, :],
                                         oqa[:, ct, j * P:(j + 1) * P],
                                         wp_bf[:, ct, cc * 512:(cc + 1) * 512],
                                         start=(ct == 0), stop=(ct == KT - 1))
                xmt = t4.tile([P, C], F32, tag="t4f32")
                nc.sync.dma_start(xmt[:], x_sh[j * P:(j + 1) * P, :])
                xm = xmt[:]
                nc.vector.tensor_tensor(xm, xm, bp_row[:, :C], ALU.add)
                dqt = t4.tile([P, C], F32, tag="t4f32")
                nc.vector.tensor_scalar(dqt[:],
                                        pp_pair[:].rearrange("p a b -> p (a b)"),
                                        rinvo_own[:, j:j + 1],
                                        None, op0=ALU.mult)
                nc.vector.tensor_tensor(xm, xm, dqt[:], ALU.add)
                nc.sync.dma_start(xmid_dram[j * P:(j + 1) * P, :], xm)
                qf = t4.tile([P, C], F32, tag="t4f32")
                ln_quant(xm, g2_row, be2_row, g2_trivial, qf[:],
                         m2_loc[:, j:j + 1])
                for ct in range(KT):
                    trq = psp.tile([P, 512], F32, tag="pb", name="trq")
                    nc.tensor.transpose(trq[:, 0:P], qf[:, ct * P:(ct + 1) * P],
                                        ident[:])
                    nc.vector.tensor_copy(q2T[:, ct, j * P:(j + 1) * P],
                                          trq[:, 0:P])

            # ---------- fc1 + gelu (sequence-parallel, ternary streamed) ----------
            rinv2c = sm.tile([P, TT_LOC], F32, name="rinv2c")
            nc.vector.tensor_scalar(rinv2c[:], m2_loc[:], mean_bc[:, 2:3],
                                    None, op0=ALU.mult)
            gqT = acts.tile([P, HKT, TOK], BF16, tag="gqk", name="gqT")
            gmax = sm.tile([P, TT_LOC], F32, name="gmax")
            nc.vector.memset(gmax[:], EPS)
            W1V = w1T[:].rearrange("(o p) h -> p o h", p=P)
            for hc in range(HC):
                bf1c = brow.tile([P, 1024], F32, tag="brow",
                                 name="bf1c")[:, :512]
                nc.sync.dma_start(
                    bf1c, bf1[None, hc * 512:(hc + 1) * 512]
                    .to_broadcast((P, 512)))
                w1c = w1p.tile([P, KT, 512], BF16, tag="w1c", name="w1c")
                nc.gpsimd.dma_start(w1c[:], W1V[:, :, hc * 512:(hc + 1) * 512])
                psf_pair = psp.tile([P, 2, 512], F32, tag="sreg",
                                    name="psf_pair")
                for tt in range(TT_LOC):
                    if tt < 2:
                        psf = psf_pair[:, tt, :]
                    else:
                        psf = psa.tile([P, 512], F32,
                                       tag=("po_e" if tt == 2 else "po_o"),
                                       name="psf")[:]
                    for ct in range(KT):
                        nc.tensor.matmul(psf, q2T[:, ct, tt * P:(tt + 1) * P],
                                         w1c[:, ct, :], start=(ct == 0),
                                         stop=(ct == KT - 1))
                    gt = t2.tile([P, 512], F32, tag="t2f32")
                    nc.scalar.activation(gt[:], psf, ACTF.Copy,
                                         scale=rinv2c[:, tt:tt + 1])
                    nc.vector.tensor_tensor(gt[:], gt[:], bf1c, ALU.add)
                    gsl = t8.tile([P, 512], BF16, tag="gact", bufs=3,
                                  name="gsl")
                    nc.scalar.activation(gsl[:], gt[:], ACTF.Gelu)
                    gpart = sm.tile([P, 1], F32, tag="gpart")
                    nc.vector.tensor_reduce(gpart[:], gsl[:], axis=AX.X,
                                            op=ALU.max,
                                            apply_absolute_value=True)
                    nc.vector.tensor_tensor(gmax[:, tt:tt + 1],
                                            gmax[:, tt:tt + 1], gpart[:],
                                            ALU.max)
                    # transpose raw gelu into fc2's lhsT layout right away
                    for qc in range(4):
                        trg = psp.tile([P, 128], BF16, tag="pb", name="trg")
                        nc.tensor.transpose(trg[:],
                                            gsl[:, qc * P:(qc + 1) * P],
                                            ident_bf[:])
                        nc.vector.tensor_copy(
                            gqT[:, hc * 4 + qc, tt * P:(tt + 1) * P], trg[:])

            # ---------- gelu quant scale row (tokens along free dim) ----------
            sg = sm.tile([P, TT_LOC], F32, name="sg")
            nc.vector.reciprocal(sg[:], gmax[:])
            nc.vector.tensor_scalar(sg[:], sg[:], 127.0, None, op0=ALU.mult)
            sgrow = t1.tile([1, TOK], F32, tag="sgrow", name="sgrow", bufs=1)
            for tt in range(TT_LOC):
                sgr_ps = psp.tile([P, 128], F32, tag="pb", name="sgr_ps")
                nc.tensor.transpose(sgr_ps[0:1, 0:P], sg[:, tt:tt + 1],
                                    ident[:])
                nc.vector.tensor_copy(sgrow[:, tt * P:(tt + 1) * P],
                                      sgr_ps[0:1, 0:P])
            gbc_ps = psp.tile([P, 512], F32, tag="pb", name="gbc_ps")
            nc.tensor.matmul(gbc_ps[:], ones_row[:], sgrow[:],
                             start=True, stop=True)
            bcs = t1.tile([P, TOK], F32, tag="bcs", name="bcs", bufs=1)
            nc.vector.tensor_copy(bcs[:], gbc_ps[:])

            # ---------- fc2 (quantize per k-tile inline, ternary streamed) ----
            f0 = psp.tile([P, 2, 512], F32, tag="sreg", name="f0")
            f1 = psp.tile([P, 2, 512], F32, tag="sreg", name="f1")
            f2a = psp.tile([P, 512], F32, tag="pb", name="f2a")
            f2b = psp.tile([P, 512], F32, tag="pb", name="f2b")
            f3a = psa.tile([P, 512], F32, tag="po_e", name="f3a")
            f3b = psa.tile([P, 512], F32, tag="po_o", name="f3b")
            fviews = [[f0[:, 0, :], f0[:, 1, :]], [f1[:, 0, :], f1[:, 1, :]],
                      [f2a[:], f2b[:]], [f3a[:], f3b[:]]]
            W2V = w2T[:].rearrange("(o p) c -> p o c", p=P)
            for kt in range(HKT):
                w2c = w2p.tile([P, C], BF16, tag="w2c", name="w2c")
                nc.gpsimd.dma_start(w2c[:], W2V[:, kt, :])
                gblk = gqT[:, kt, :]
                nc.vector.tensor_tensor(gblk, gblk, bcs[:], ALU.mult)
                nc.vector.tensor_scalar(gblk, gblk, MAGIC, MAGIC,
                                        op0=ALU.add, op1=ALU.subtract)
                for tt in range(TT_LOC):
                    for cc in range(2):
                        nc.tensor.matmul(
                            fviews[tt][cc],
                            gqT[:, kt, tt * P:(tt + 1) * P],
                            w2c[:, cc * 512:(cc + 1) * 512],
                            start=(kt == 0), stop=(kt == HKT - 1),
                            skip_group_check=True)

            # ---------- final: y = x_mid + deq(fc2) + bf2 ----------
            bf2_row = bcast_row(bf2[:], C, "bf2_row", pool=brow)
            for tt in range(TT_LOC):
                xmr = t4.tile([P, C], F32, tag="t4f32")
                nc.sync.dma_start(xmr[:], xmid_dram[tt * P:(tt + 1) * P, :])
                deqf = sm.tile([P, 1], F32, tag="deqf")
                nc.vector.tensor_scalar(deqf[:], gmax[:, tt:tt + 1],
                                        mean_bc[:, 3:4], None, op0=ALU.mult)
                for cc in range(2):
                    yt = t2.tile([P, 512], F32, tag="t2f32")
                    nc.vector.tensor_scalar(yt[:], fviews[tt][cc], deqf[:],
                                            None, op0=ALU.mult)
                    nc.vector.tensor_tensor(
                        yt[:], yt[:], bf2_row[:, cc * 512:(cc + 1) * 512],
                        ALU.add)
                    nc.vector.tensor_tensor(
                        yt[:], yt[:], xmr[:, cc * 512:(cc + 1) * 512],
                        ALU.add)
                    nc.sync.dma_start(
                        y_sh[tt * P:(tt + 1) * P, cc * 512:(cc + 1) * 512],
                        yt[:])

    nc.compile()
    return nc


_CACHE = {}


def _ternarize(w):
    beta = np.float32(max(np.mean(np.abs(w), dtype=np.float64), EPS))
    q = np.clip(np.rint(w * (np.float32(1.0) / beta)), -1.0, 1.0)
    return q.astype(BF16NP), beta


def kernel(**inputs):
    m = _imports()
    x = np.ascontiguousarray(np.asarray(inputs["x"]), dtype=np.float32)
    assert int(inputs["num_heads"]) == H
    w_qkv = np.asarray(inputs["w_qkv"], np.float32)
    b_qkv = np.asarray(inputs["b_qkv"], np.float32)
    w_proj = np.asarray(inputs["w_proj"], np.float32)
    b_proj = np.asarray(inputs["b_proj"], np.float32)
    w_fc1 = np.asarray(inputs["w_fc1"], np.float32)
    b_fc1 = np.asarray(inputs["b_fc1"], np.float32)
    w_fc2 = np.asarray(inputs["w_fc2"], np.float32)
    b_fc2 = np.asarray(inputs["b_fc2"], np.float32)
    g1 = np.asarray(inputs["g1"], np.float32)
    be1 = np.asarray(inputs["be1"], np.float32)
    g2 = np.asarray(inputs["g2"], np.float32)
    be2 = np.asarray(inputs["be2"], np.float32)

    g1_trivial = bool(np.all(g1 == 1.0) and np.all(be1 == 0.0))
    g2_trivial = bool(np.all(g2 == 1.0) and np.all(be2 == 0.0))

    key = (g1_trivial, g2_trivial)
    if key not in _CACHE:
        _CACHE[key] = build_kernel(g1_trivial, g2_trivial)
    nc = _CACHE[key]

    tq_qkv, beta_qkv = _ternarize(w_qkv)
    tq_proj, beta_proj = _ternarize(w_proj)
    tq_fc1, beta_fc1 = _ternarize(w_fc1)
    tq_fc2, beta_fc2 = _ternarize(w_fc2)
    scl = np.array([beta_qkv, beta_proj, beta_fc1, beta_fc2],
                   np.float32) / np.float32(127.0)
    w1T_full = np.ascontiguousarray(tq_fc1.T)
    w2T_full = np.ascontiguousarray(tq_fc2.T)
    wpT_full = np.ascontiguousarray(tq_proj.T)

    in_maps = []
    for c in range(NCORES):
        g, r = divmod(c, TP)
        tok = slice(TOK * r, TOK * (r + 1))
        hsl = slice(CS * r, CS * (r + 1))
        im = {
            "x_sh": np.ascontiguousarray(x[g, tok]),
            "x_full": np.ascontiguousarray(x[g]),
            "wqkT": np.ascontiguousarray(
                np.concatenate([tq_qkv[hsl, :].T,
                                tq_qkv[C:2 * C][hsl, :].T], axis=1)),
            "wvT": np.ascontiguousarray(tq_qkv[2 * C:][hsl, :].T),
            "wpT": wpT_full,
            "w1T": w1T_full,
            "w2T": w2T_full,
            "scl": scl,
            "bqk": np.ascontiguousarray(
                np.concatenate([b_qkv[hsl], b_qkv[C:][hsl]])),
            "bv": np.ascontiguousarray(b_qkv[2 * C:][hsl]),
            "bp": b_proj,
            "onehot": np.eye(TP, dtype=np.float32)[r],
            "bf1": b_fc1,
            "bf2": b_fc2,
        }
        if not g1_trivial:
            im["g1"], im["be1"] = g1, be1
        if not g2_trivial:
            im["g2"], im["be2"] = g2, be2
        in_maps.append(im)

    global _last_in_maps
    _last_in_maps = in_maps
    res = m["run"](nc, in_maps, core_ids=list(range(NCORES)))
    out = np.empty((B, N, C), np.float32)
    for c in range(NCORES):
        g, r = divmod(c, TP)
        out[g, TOK * r:TOK * (r + 1)] = res.results[c]["y_sh"]
    return out
